# revision 62
# baseline (speedup 1.0000x reference)
"""Multi-head attention (B=4, S=2048, D=1024, H=16, Dk=64) on 8 trn2 NeuronCores.

Sharding: core = (batch b, head-group g), b in 0..3, g in 0..1.  Each core
computes attention for its batch and its 8 heads plus the partial out
projection for its 512 rows of Wo; host sums the two partials per batch and
adds bo.

Key optimizations over the naive version:
  - Host-side key compaction: mask keys (~50% zeros) are gathered out of k/v
    before upload, so the device only scores/exps/attends over valid keys
    (padded to a multiple of 128; pad lanes get a -1e9 exp bias -> probs 0).
  - Host-side transposes + bf16 casts: q/k/v arrive as [D, S] bf16, so phase A
    needs no PE transposes, weight loads use FWL, and DMA bytes halve.
  - Scores for a head PAIR run as two concurrent row-tiled K=64 matmuls
    (partition bases 0/64) into one PSUM tile, so a single [128,1024] ACT exp
    covers both heads (ACT is the bottleneck engine; fewer+wider ACTIVATEs).
  - The ones-column in vh yields softmax denominators for free (row 64 of the
    attn PSUM); reciprocal via the fast approx DVE op; recip broadcast across
    64 partitions with a K=1 matmul; normalize straight out of PSUM.
  - Software-pipelined emission (engine queues are FIFO): scores(sk+1) is
    emitted before attn(sk); normalization of the previous pair and the out
    projection of the previous query block are emitted as PE fillers early in
    the next pair's key loop.
"""

import sys

sys.path.insert(0, "/opt/trn_rl_repo")

import numpy as np

B, S, D, H, DK = 4, 2048, 1024, 16, 64
CPG = 512          # projection columns per core (8 heads x 64)
NPAIR = 4          # head pairs per core
NDCH = D // 128    # contraction chunks for projections
NCORES = 8

_cache = {}


def _build_nc(nskv, zero_bias):
    import contextlib

    import concourse.bass as bass
    import concourse.tile as tile
    from concourse import bacc, mybir

    f32 = mybir.dt.float32
    bf16 = mybir.dt.bfloat16
    Exp = mybir.ActivationFunctionType.Exp

    skv = nskv * 128

    nc = bacc.Bacc("TRN2", target_bir_lowering=False, debug=False)

    qt_d = nc.dram_tensor("qt", [D, S], bf16, kind="ExternalInput").ap()
    kt_d = nc.dram_tensor("kt", [D, skv], bf16, kind="ExternalInput").ap()
    vt_d = nc.dram_tensor("vt", [D, skv], bf16, kind="ExternalInput").ap()
    wq_d = nc.dram_tensor("wq", [D, CPG], bf16, kind="ExternalInput").ap()
    wk_d = nc.dram_tensor("wk", [D, CPG], bf16, kind="ExternalInput").ap()
    wv_d = nc.dram_tensor("wv", [D, CPG], bf16, kind="ExternalInput").ap()
    wo_d = nc.dram_tensor("wo", [CPG, D], bf16, kind="ExternalInput").ap()
    mb_d = nc.dram_tensor("maskbias", [128, nskv], f32, kind="ExternalInput").ap()
    ones_d = nc.dram_tensor("ones", [128, 512], bf16, kind="ExternalInput").ap()
    ident_d = nc.dram_tensor("ident", [128, 128], bf16, kind="ExternalInput").ap()
    if not zero_bias:
        bq_d = nc.dram_tensor("bq", [128, NPAIR], f32, kind="ExternalInput").ap()
        bk_d = nc.dram_tensor("bk", [128, NPAIR], f32, kind="ExternalInput").ap()
        bv_d = nc.dram_tensor("bv", [1, CPG], bf16, kind="ExternalInput").ap()
    out_d = nc.dram_tensor("out", [S, D], bf16, kind="ExternalOutput").ap()

    # skv split into 512-wide column chunks for the khT projection copies
    kq_chunks = []
    o = 0
    while o < skv:
        w = min(512, skv - o)
        kq_chunks.append((o, w))
        o += w

    with tile.TileContext(nc) as tc:
        with contextlib.ExitStack() as ctx:
            # ---------- persistent tensors + constants ----------
            persist = ctx.enter_context(tc.tile_pool(name="persist", bufs=1))
            consts = ctx.enter_context(tc.tile_pool(name="consts", bufs=1))

            qhT_sb = persist.tile([128, NPAIR, S], bf16)      # [c%128, pair, sq]
            khT_sb = persist.tile([128, NPAIR, skv], bf16)    # [c%128, pair, sk]
            vh_sb = persist.tile([128, nskv, 8, 128], bf16)  # [vdims|ones|zeros]
            concatT_sb = persist.tile([128, NPAIR, S], bf16)

            ones_sb = consts.tile([1, 512], bf16)
            nc.gpsimd.memset(ones_sb, 1.0)
            swdge_warm = consts.tile([1, 16], bf16)
            nc.gpsimd.dma_start(out=swdge_warm, in_=ones_sb[0:1, 0:16])
            # wake the other two DMA queues immediately too: first packets
            # otherwise take ~6us to land after the kernel starts
            sy_warm = consts.tile([1, 16], bf16)
            nc.sync.dma_start(out=sy_warm, in_=ones_d[0:1, 0:16])
            sc_warm = consts.tile([1, 16], bf16)
            nc.scalar.dma_start(out=sc_warm, in_=ones_d[0:1, 16:32])
            lib_warm = consts.tile([2, 16], f32)
            nc.gpsimd.memset(lib_warm[0:1, :], 1.0)
            nc.gpsimd.partition_broadcast(lib_warm, lib_warm[0:1, :])
            # prewarm the ACT exp table so the first scores exp doesn't pay
            # the table load
            act_warm = consts.tile([2, 16], bf16)
            nc.scalar.activation(out=act_warm, in_=lib_warm, func=Exp, scale=0.125)
            mb_sb = consts.tile([128, nskv], f32)
            nc.gpsimd.dma_start(out=mb_sb, in_=mb_d)
            ident_sb = consts.tile([128, 128], bf16)
            nc.gpsimd.dma_start(out=ident_sb, in_=ident_d)
            wo_sb = consts.tile([128, NPAIR, D], bf16)

            if not zero_bias:
                bq_sb = consts.tile([128, NPAIR], f32)
                nc.sync.dma_start(out=bq_sb, in_=bq_d)
                bk_sb = consts.tile([128, NPAIR], f32)
                nc.sync.dma_start(out=bk_sb, in_=bk_d)
                bv_sb = consts.tile([1, CPG], bf16)
                nc.sync.dma_start(out=bv_sb, in_=bv_d)

            # ---------- phase A: projections (no transposes needed) ----------
            xpool = ctx.enter_context(tc.tile_pool(name="xpool", bufs=1))
            wpool = ctx.enter_context(tc.tile_pool(name="wpool", bufs=1))
            with contextlib.ExitStack() as actx:
                prpool = actx.enter_context(
                    tc.tile_pool(name="prpool", bufs=8, space="PSUM")
                )

                # DMA schedule: wk first on sync (kproj needs all of it), kt
                # column-chunk-major on scalar (kproj's first output chunk
                # needs cols [0:512] of every row chunk), vt early on the
                # otherwise-idle gpsimd queue, everything else behind.
                kt_sb = xpool.tile([128, NDCH, skv], bf16, tag="kt")
                wk_sb = wpool.tile([128, NDCH, CPG], bf16, tag="wk")
                vt_sb = xpool.tile([128, NDCH, skv], bf16, tag="vt")
                wv_sb = wpool.tile([128, NDCH, CPG], bf16, tag="wv")
                qt_sb = xpool.tile([128, NDCH, S], bf16, tag="qt")
                wq_sb = wpool.tile([128, NDCH, CPG], bf16, tag="wq")
                def split_q(j):
                    return nc.scalar if j < 4 else nc.gpsimd

                for j in range(NDCH):
                    nc.sync.dma_start(out=wk_sb[:, j, :], in_=wk_d[j * 128 : j * 128 + 128, :])
                for o, w in ((0, 512), (512, skv - 512)) if skv > 512 else ((0, skv),):
                    for j in range(NDCH):
                        split_q(j).dma_start(
                            out=kt_sb[:, j, o : o + w],
                            in_=kt_d[j * 128 : j * 128 + 128, o : o + w],
                        )
                for j in range(NDCH):
                    nc.sync.dma_start(out=wv_sb[:, j, :], in_=wv_d[j * 128 : j * 128 + 128, :])
                for j in range(NDCH):
                    nc.sync.dma_start(out=wq_sb[:, j, :], in_=wq_d[j * 128 : j * 128 + 128, :])
                for j in range(NDCH):
                    split_q(j).dma_start(
                        out=qt_sb[:, j, 0:512], in_=qt_d[j * 128 : j * 128 + 128, 0:512]
                    )
                for j in range(NDCH):
                    split_q(j).dma_start(out=vt_sb[:, j, :], in_=vt_d[j * 128 : j * 128 + 128, :])
                # only the ones column needs initializing: the attn matmuls
                # read vh[..., 0:DK+1], so the rest of vh can stay garbage
                nc.gpsimd.memset(vh_sb[:, :, :, DK], 1.0)
                for j in range(NDCH):
                    nc.sync.dma_start(
                        out=qt_sb[:, j, 512:1024], in_=qt_d[j * 128 : j * 128 + 128, 512:1024]
                    )
                for j in range(NPAIR):
                    nc.sync.dma_start(
                        out=wo_sb[:, j, :], in_=wo_d[j * 128 : j * 128 + 128, :]
                    )
                for j in range(NDCH):
                    nc.sync.dma_start(
                        out=qt_sb[:, j, 1024:S], in_=qt_d[j * 128 : j * 128 + 128, 1024:S]
                    )

                # HAM warmup: the PE clock-gate needs ~3.4us of sustained
                # activity to reach full clock; burn the initial DMA wait on
                # dummy matmuls over constant data so phase A starts warm
                warm_ps = prpool.tile([128, 512], f32, tag="pr", name="warmps")
                for w in range(6):
                    nc.tensor.matmul(
                        warm_ps,
                        lhsT=ones_sb[0:1, 0:128],
                        rhs=ones_sb[0:1, :],
                        start=(w == 0),
                        stop=(w == 5),
                    )

                # k projection: khT[c, sk] = Wk^T @ kT
                for o, w in kq_chunks:
                    prs = [prpool.tile([128, 512], f32, tag="pr", name=f"pr{i}") for i in range(4)]
                    for j in range(NDCH):
                        for cch in range(4):
                            nc.tensor.matmul(
                                prs[cch][:, 0:w],
                                lhsT=wk_sb[:, j, cch * 128 : cch * 128 + 128],
                                rhs=kt_sb[:, j, o : o + w],
                                start=(j == 0),
                                stop=(j == NDCH - 1),
                            )
                    for cch in range(4):
                        if zero_bias:
                            nc.vector.tensor_copy(
                                out=khT_sb[:, cch, o : o + w], in_=prs[cch][:, 0:w]
                            )
                        else:
                            nc.vector.tensor_scalar_add(
                                khT_sb[:, cch, o : o + w],
                                prs[cch][:, 0:w],
                                bk_sb[:, cch : cch + 1],
                            )

                # v projection: vh[sk, c] = vT^T @ Wv -- its PE work largely
                # hides under the input-DMA ramp
                for skc in range(nskv):
                    vpr = prpool.tile([128, 512], f32, tag="pr", name="vpr")
                    for j in range(NDCH):
                        if not zero_bias and j == 0:
                            nc.tensor.matmul(
                                vpr,
                                lhsT=ones_sb[0:1, 0:128],
                                rhs=bv_sb[0:1, :],
                                start=True,
                                stop=False,
                            )
                        nc.tensor.matmul(
                            vpr,
                            lhsT=vt_sb[:, j, skc * 128 : skc * 128 + 128],
                            rhs=wv_sb[:, j, :],
                            start=(zero_bias and j == 0),
                            stop=(j == NDCH - 1),
                        )
                    nc.vector.tensor_copy(
                        out=vh_sb[:, skc, :, 0:DK],
                        in_=vpr.rearrange("p (h d) -> p h d", h=8),
                    )

                # q projection, block 0, pairs 0-1 only (pairs 2-3 and blocks
                # 1-3 run as phase-B fillers -- attention starts sooner)
                for sqq in range(1):
                    o = sqq * 512
                    prs = [prpool.tile([128, 512], f32, tag="pr", name=f"pr{i}") for i in range(2)]
                    for j in range(NDCH):
                        for cch in range(2):
                            nc.tensor.matmul(
                                prs[cch],
                                lhsT=wq_sb[:, j, cch * 128 : cch * 128 + 128],
                                rhs=qt_sb[:, j, o : o + 512],
                                start=(j == 0),
                                stop=(j == NDCH - 1),
                            )
                    for cch in range(2):
                        if zero_bias:
                            nc.vector.tensor_copy(
                                out=qhT_sb[:, cch, o : o + 512], in_=prs[cch]
                            )
                        else:
                            nc.vector.tensor_scalar_add(
                                qhT_sb[:, cch, o : o + 512],
                                prs[cch],
                                bq_sb[:, cch : cch + 1],
                            )

            # ---------- phase B + C: attention, fused with out projection ----
            with contextlib.ExitStack() as bctx:
                probpool = bctx.enter_context(tc.tile_pool(name="probpool", bufs=4))
                dnpool = bctx.enter_context(tc.tile_pool(name="dnpool", bufs=3))
                rc32pool = bctx.enter_context(tc.tile_pool(name="rc32pool", bufs=3))
                rc16pool = bctx.enter_context(tc.tile_pool(name="rc16pool", bufs=2))
                outpool = bctx.enter_context(tc.tile_pool(name="outpool", bufs=3))
                scpool = bctx.enter_context(
                    tc.tile_pool(name="scpool", bufs=2, space="PSUM")
                )
                atpool = bctx.enter_context(
                    tc.tile_pool(name="atpool", bufs=2, space="PSUM")
                )
                auxpool = bctx.enter_context(
                    tc.tile_pool(name="auxpool", bufs=2, space="PSUM")
                )

                def emit_norm_fast_pre(atA, atB):
                    """Tail norm, part 1: atf copies + lane-parallel recip +
                    bf16 cast + reshape back to rows.  DMAs ride the idle
                    sync queue (gpsimd still owes norm(14)'s work)."""
                    heads = ((0, atA), (1, atB))
                    atfs, rfulls = {}, {}
                    for hh, at in heads:
                        atf = rc32pool.tile([128, 512], f32, tag="atf")
                        nc.vector.tensor_copy(out=atf[0:65, :], in_=at[0:65, :])
                        atfs[hh] = atf
                    r4bs = {}
                    for hh, at in heads:
                        dn4 = dnpool.tile([4, 128], f32, tag="dn4")
                        nc.sync.dma_start(out=dn4, in_=atfs[hh][64:65, :])
                        r4 = dnpool.tile([4, 128], f32, tag="r4")
                        nc.vector.reciprocal(r4, dn4)
                        r4b = dnpool.tile([4, 128], bf16, tag="r4b")
                        nc.vector.tensor_copy(out=r4b, in_=r4)
                        r4bs[hh] = r4b
                    for hh, at in heads:
                        rfull = dnpool.tile([1, 512], bf16, tag="rfull")
                        nc.sync.dma_start(out=rfull, in_=r4bs[hh])
                        rfulls[hh] = rfull
                    return atfs, rfulls

                def emit_norm_fast_post(sqb, pair, atfs, rfulls):
                    """Tail norm, part 2: 64-partition broadcast as a K=1
                    matmul on the (tail-idle) PE, then the normalize mul."""
                    q0 = sqb * 512
                    reps = {}
                    for hh in (0, 1):
                        rep = scpool.tile([64, 512], f32, tag="sc", name="rep")
                        nc.tensor.matmul(
                            rep,
                            lhsT=ones_sb[0:1, 0:64],
                            rhs=rfulls[hh],
                            start=True,
                            stop=True,
                        )
                        reps[hh] = rep
                    for hh in (0, 1):
                        base = hh * 64
                        nc.vector.tensor_mul(
                            concatT_sb[base : base + 64, pair, q0 : q0 + 512],
                            atfs[hh][0:64, :],
                            reps[hh],
                        )

                def emit_norm(sqb, pair, atA, atB):
                    """Normalize both heads of a finished pair into concatT.

                    The PSUM->SBUF copy comes first so the attn PSUM slot is
                    released ~0.7us after the last attn matmul.  The [1,512]
                    denominator row is reshaped to [4,128] by DMA so the DVE
                    reciprocal runs lane-parallel (0.8us instead of 3.3us),
                    DMA'd back to a row, broadcast across 64 partitions and
                    multiplied on the otherwise-idle GPSIMD engine -- off
                    every critical queue."""
                    q0 = sqb * 512
                    heads = ((0, atA), (1, atB))
                    atfs, r4s, reps = {}, {}, {}
                    for hh, at in heads:
                        atf = rc32pool.tile([128, 512], f32, tag="atf")
                        nc.vector.tensor_copy(out=atf[0:65, :], in_=at[0:65, :])
                        atfs[hh] = atf
                    for hh, at in heads:
                        dn4 = dnpool.tile([4, 128], f32, tag="dn4")
                        nc.gpsimd.dma_start(out=dn4, in_=atfs[hh][64:65, :])
                        r4 = dnpool.tile([4, 128], f32, tag="r4")
                        nc.vector.reciprocal(r4, dn4)
                        r4s[hh] = r4
                    for hh, at in heads:
                        rfull = dnpool.tile([1, 512], f32, tag="rfull")
                        nc.gpsimd.dma_start(out=rfull, in_=r4s[hh])
                        rep = rc16pool.tile([64, 512], f32, tag="rep")
                        nc.gpsimd.partition_broadcast(rep, rfull)
                        reps[hh] = rep
                    for hh, at in heads:
                        base = hh * 64
                        nc.vector.tensor_mul(
                            concatT_sb[base : base + 64, pair, q0 : q0 + 512],
                            atfs[hh][0:64, :],
                            reps[hh],
                        )

                # staging for the last query block's partial out projection
                # (pairs 0-1 accumulated as their norms land; pairs 2-3 are
                # re-accumulated in PSUM at the tail via an identity matmul)
                stage_sb = persist.tile([128, 8, 512], bf16, name="stage")

                # ---- filler quanta: each closure emits EXACTLY ONE matmul
                # (~226ns of PE) so a filler never delays the next scores pair
                # past the ACT pace (~1.1us/step, ~340ns/step of PE slack).
                fillers = []
                aux_live = {}

                def q_out_mm(sqb, sqc, do, j, jmax=NPAIR - 1, dmaq=None):
                    def f():
                        q0 = sqb * 512 + sqc * 128
                        key = ("o", sqb, sqc, do)
                        if j == 0:
                            aux_live[key] = auxpool.tile(
                                [128, 512], f32, tag="aux", name="ops"
                            )
                        ops = aux_live[key]
                        nc.tensor.matmul(
                            ops,
                            lhsT=concatT_sb[:, j, q0 : q0 + 128],
                            rhs=wo_sb[:, j, do * 512 : do * 512 + 512],
                            start=(j == 0),
                            stop=(j == jmax),
                        )
                        if j == jmax:
                            del aux_live[key]
                            if jmax == NPAIR - 1:
                                osb = outpool.tile([128, 512], bf16, tag="osb")
                                nc.vector.tensor_copy(out=osb, in_=ops)
                                (dmaq or nc.sync).dma_start(
                                    out=out_d[
                                        q0 : q0 + 128, do * 512 : do * 512 + 512
                                    ],
                                    in_=osb,
                                )
                            else:
                                # partial for the last query block -> staging
                                nc.vector.tensor_copy(
                                    out=stage_sb[:, sqc * 2 + do, :], in_=ops
                                )
                    return f

                def final3_a(g, sqc, do):
                    """Tail out-projection chunk, part 1: reload the staged
                    pair-0/1 partial into PSUM via an identity matmul, then
                    accumulate the pair-2 term.  Runs under the norm chain."""
                    q0 = 3 * 512 + sqc * 128
                    pool, tag = (atpool, "at") if g % 2 else (auxpool, "aux")
                    ops = pool.tile([128, 512], f32, tag=tag, name="f3")
                    nc.tensor.matmul(
                        ops,
                        lhsT=ident_sb,
                        rhs=stage_sb[:, sqc * 2 + do, :],
                        start=True,
                        stop=False,
                    )
                    nc.tensor.matmul(
                        ops,
                        lhsT=concatT_sb[:, 2, q0 : q0 + 128],
                        rhs=wo_sb[:, 2, do * 512 : do * 512 + 512],
                        start=False,
                        stop=False,
                    )
                    return ops

                def final3_b(g, sqc, do, ops):
                    """Part 2: pair-3 term (waits the final norm), PSUM->SBUF
                    cast on the post-exp-idle scalar engine, store."""
                    q0 = 3 * 512 + sqc * 128
                    nc.tensor.matmul(
                        ops,
                        lhsT=concatT_sb[:, 3, q0 : q0 + 128],
                        rhs=wo_sb[:, 3, do * 512 : do * 512 + 512],
                        start=False,
                        stop=True,
                    )
                    # keep the gpsimd queue quiet here: its end-of-program
                    # drain takes ~8us and only overlaps the tail if gpsimd's
                    # instruction stream ends early
                    osb = outpool.tile([128, 512], bf16, tag="osb")
                    nc.scalar.copy(out=osb, in_=ops)
                    dmaq = (nc.sync, nc.scalar)[g % 2]
                    dmaq.dma_start(
                        out=out_d[q0 : q0 + 128, do * 512 : do * 512 + 512],
                        in_=osb,
                    )

                def g_kproj(o, w, cch):
                    """Group-atomic deferred k-projection chunk: one aux tile,
                    8 contraction matmuls, copy to khT.  Atomic so its PSUM
                    slot never interleaves with another in-flight group."""
                    def f():
                        pr = auxpool.tile([128, 512], f32, tag="aux", name="kpr")
                        for j in range(NDCH):
                            nc.tensor.matmul(
                                pr[:, 0:w],
                                lhsT=wk_sb[:, j, cch * 128 : cch * 128 + 128],
                                rhs=kt_sb[:, j, o : o + w],
                                start=(j == 0),
                                stop=(j == NDCH - 1),
                            )
                        if zero_bias:
                            nc.vector.tensor_copy(
                                out=khT_sb[:, cch, o : o + w], in_=pr[:, 0:w]
                            )
                        else:
                            nc.vector.tensor_scalar_add(
                                khT_sb[:, cch, o : o + w],
                                pr[:, 0:w],
                                bk_sb[:, cch : cch + 1],
                            )
                    return f

                def g_qproj(sqq, cch):
                    """Group-atomic deferred q-projection chunk."""
                    def f():
                        o = sqq * 512
                        pr = auxpool.tile([128, 512], f32, tag="aux", name="qpg")
                        for j in range(NDCH):
                            nc.tensor.matmul(
                                pr,
                                lhsT=wq_sb[:, j, cch * 128 : cch * 128 + 128],
                                rhs=qt_sb[:, j, o : o + 512],
                                start=(j == 0),
                                stop=(j == NDCH - 1),
                            )
                        if zero_bias:
                            nc.vector.tensor_copy(
                                out=qhT_sb[:, cch, o : o + 512], in_=pr
                            )
                        else:
                            nc.vector.tensor_scalar_add(
                                qhT_sb[:, cch, o : o + 512],
                                pr,
                                bq_sb[:, cch : cch + 1],
                            )
                    return f

                def q_qproj_mm(sqq, cch, j):
                    def f():
                        o = sqq * 512
                        key = ("q", sqq, cch)
                        if j == 0:
                            aux_live[key] = auxpool.tile(
                                [128, 512], f32, tag="aux", name="qpr"
                            )
                        pr = aux_live[key]
                        nc.tensor.matmul(
                            pr,
                            lhsT=wq_sb[:, j, cch * 128 : cch * 128 + 128],
                            rhs=qt_sb[:, j, o : o + 512],
                            start=(j == 0),
                            stop=(j == NDCH - 1),
                        )
                        if j == NDCH - 1:
                            del aux_live[key]
                            if zero_bias:
                                nc.vector.tensor_copy(
                                    out=qhT_sb[:, cch, o : o + 512], in_=pr
                                )
                            else:
                                nc.vector.tensor_scalar_add(
                                    qhT_sb[:, cch, o : o + 512],
                                    pr,
                                    bq_sb[:, cch : cch + 1],
                                )
                    return f

                # ---- flat (block, sk) software-pipelined stream ----
                blocks = [(sqb, pair) for sqb in range(4) for pair in range(NPAIR)]
                steps = [(bi, sk) for bi in range(len(blocks)) for sk in range(nskv)]
                at_tiles = [None] * len(blocks)
                probs_live = {}

                def emit_scores_exp(i):
                    bi, sk = steps[i]
                    sqb, pair = blocks[bi]
                    q0 = sqb * 512
                    sc = scpool.tile([128, 1024], f32, tag="sc")
                    nc.tensor.matmul(
                        sc[:, 0:512],
                        lhsT=khT_sb[0:64, pair, sk * 128 : sk * 128 + 128],
                        rhs=qhT_sb[0:64, pair, q0 : q0 + 512],
                        start=True,
                        stop=True,
                    )
                    nc.tensor.matmul(
                        sc[:, 512:1024],
                        lhsT=khT_sb[64:128, pair, sk * 128 : sk * 128 + 128],
                        rhs=qhT_sb[64:128, pair, q0 : q0 + 512],
                        start=True,
                        stop=True,
                    )
                    probs = probpool.tile([128, 1024], bf16, tag="probs")
                    probs_live[i] = probs
                    nc.scalar.activation(
                        out=probs,
                        in_=sc,
                        func=Exp,
                        bias=mb_sb[:, sk : sk + 1],
                        scale=0.125,
                    )

                def emit_vproj(skc):
                    """JIT v projection for key chunk skc (phase-B block 0):
                    vh[skc, c] = vT^T @ Wv."""
                    pr = auxpool.tile([128, 512], f32, tag="aux", name="vpr")
                    for j in range(NDCH):
                        if not zero_bias and j == 0:
                            nc.tensor.matmul(
                                pr,
                                lhsT=ones_sb[0:1, 0:128],
                                rhs=bv_sb[0:1, :],
                                start=True,
                                stop=False,
                            )
                        nc.tensor.matmul(
                            pr,
                            lhsT=vt_sb[:, j, skc * 128 : skc * 128 + 128],
                            rhs=wv_sb[:, j, :],
                            start=(zero_bias and j == 0),
                            stop=(j == NDCH - 1),
                        )
                    nc.vector.tensor_copy(
                        out=vh_sb[:, skc, :, 0:DK],
                        in_=pr.rearrange("p (h d) -> p h d", h=8),
                    )

                def emit_attn(i):
                    bi, sk = steps[i]
                    sqb, pair = blocks[bi]
                    hA, hB = 2 * pair, 2 * pair + 1
                    if sk == 0:
                        at_tiles[bi] = (
                            atpool.tile([128, 512], f32, tag="at", name="atA"),
                            atpool.tile([128, 512], f32, tag="at", name="atB"),
                        )
                    atA, atB = at_tiles[bi]
                    probs = probs_live.pop(i)
                    nc.tensor.matmul(
                        atA[0 : DK + 1, :],
                        lhsT=vh_sb[:, sk, hA, 0 : DK + 1],
                        rhs=probs[:, 0:512],
                        start=(sk == 0),
                        stop=(sk == nskv - 1),
                    )
                    nc.tensor.matmul(
                        atB[0 : DK + 1, :],
                        lhsT=vh_sb[:, sk, hB, 0 : DK + 1],
                        rhs=probs[:, 512:1024],
                        start=(sk == 0),
                        stop=(sk == nskv - 1),
                    )

                def on_attn_done(bi):
                    sqb, pair = blocks[bi]
                    if bi == len(blocks) - 1:
                        return  # tail: fast norm handled in the drain
                    emit_norm(sqb, pair, *at_tiles[bi])
                    if pair == NPAIR - 1 and sqb < 3:
                        for sqc in range(4):
                            for do in range(2):
                                for j in range(NPAIR):
                                    fillers.append(q_out_mm(sqb, sqc, do, j))
                    if bi == len(blocks) - 3:
                        # norm(13) emitted: pairs 0-1 of sqb3 are final;
                        # start their out-projection partial into staging
                        for sqc in range(4):
                            for do in range(2):
                                for j in range(2):
                                    fillers.append(
                                        q_out_mm(3, sqc, do, j, jmax=1)
                                    )
                    # (pair-2 terms of the last query block run in the drain,
                    # after norm15's vector ops are queued)

                LOOK = 2
                for i in range(len(steps)):
                    bi, sk = steps[i]
                    if i == 0:
                        for cch in (2, 3):
                            for j in range(NDCH):
                                fillers.append(q_qproj_mm(0, cch, j))
                    if sk == 0 and (
                        bi == 1 or (bi % NPAIR == 0 and blocks[bi][0] in (1, 2))
                    ):
                        sqq = 1 if bi == 1 else blocks[bi][0] + 1
                        for cch in range(4):
                            for j in range(NDCH):
                                fillers.append(q_qproj_mm(sqq, cch, j))
                    emit_scores_exp(i)
                    if i >= LOOK:
                        emit_attn(i - LOOK)
                        bj, skj = steps[i - LOOK]
                        if skj == nskv - 1:
                            on_attn_done(bj)
                    remaining = len(steps) - i
                    npop = 2 if len(fillers) > min(24, remaining) else 1
                    for _ in range(npop):
                        if fillers:
                            fillers.pop(0)()

                # ---- drain ----
                for i in range(max(len(steps) - LOOK, 0), len(steps)):
                    emit_attn(i)
                    bj, skj = steps[i]
                    if skj == nskv - 1:
                        on_attn_done(bj)
                # final norm's vector/gpsimd chain first; PE fillers run
                # underneath it, then the PE-side broadcast + mul, then the
                # last 8 out-projection chunks (pair-3 term + staged partial)
                atfs, rfulls = emit_norm_fast_pre(*at_tiles[len(blocks) - 1])
                while fillers:
                    fillers.pop(0)()
                chunks = [(sqc, do) for sqc in range(4) for do in range(2)]
                ops_live = {}
                # first 4 chunks' stage-reload + pair-2 terms run under the
                # norm chain (at/aux pools give 4 tiles in flight)
                for g in range(4):
                    ops_live[g] = final3_a(g, *chunks[g])
                # a few warm matmuls bridge to the reciprocal/reshape window
                # so HAM stays at full clock
                for w in range(8):
                    warm = scpool.tile([128, 1024], f32, tag="sc", name=f"wm{w}")
                    nc.tensor.matmul(
                        warm[:, 0:512],
                        lhsT=khT_sb[0:64, 0, 0:128],
                        rhs=qhT_sb[0:64, 0, 0:512],
                        start=True,
                        stop=True,
                    )
                emit_norm_fast_post(3, 3, atfs, rfulls)
                for g in range(8):
                    final3_b(g, *chunks[g], ops_live.pop(g))
                    if g + 4 < 8:
                        ops_live[g + 4] = final3_a(g + 4, *chunks[g + 4])

    nc.compile()
    return nc


def get_nc(nskv=9, zero_bias=True):
    key = (nskv, zero_bias)
    if key not in _cache:
        _cache[key] = _build_nc(nskv, zero_bias)
    return _cache[key]


def make_in_maps(q, k, v, mask, Wq, bq, Wk, bk, Wv, bv, Wo, bo):
    import ml_dtypes

    f32 = np.float32
    bf16 = ml_dtypes.bfloat16
    c = np.ascontiguousarray

    mask = np.asarray(mask)
    idxs = [np.nonzero(mask[b, 0] != 0)[0] for b in range(B)]
    kvs = [len(ix) for ix in idxs]
    nskv = max(1, (max(kvs) + 127) // 128)
    skv = nskv * 128

    zero_bias = (
        not np.any(np.asarray(bq))
        and not np.any(np.asarray(bk))
        and not np.any(np.asarray(bv))
    )

    Wq, Wk, Wv, Wo = (np.asarray(a, f32) for a in (Wq, Wk, Wv, Wo))

    in_maps = []
    for core in range(NCORES):
        b, g = core // 2, core % 2
        cols = slice(g * CPG, (g + 1) * CPG)
        ix = idxs[b]
        kv = kvs[b]

        kc = np.zeros((skv, D), f32)
        vc = np.zeros((skv, D), f32)
        kc[:kv] = np.asarray(k[b], f32)[ix]
        vc[:kv] = np.asarray(v[b], f32)[ix]

        mbflat = np.where(np.arange(skv) < kv, 0.0, -1e9).astype(f32)

        m = {
            "qt": c(np.asarray(q[b], f32).T.astype(bf16)),
            "kt": c(kc.T.astype(bf16)),
            "vt": c(vc.T.astype(bf16)),
            "wq": c(Wq[:, cols].astype(bf16)),
            "wk": c(Wk[:, cols].astype(bf16)),
            "wv": c(Wv[:, cols].astype(bf16)),
            "wo": c(Wo[cols, :].astype(bf16)),
            "maskbias": c(mbflat.reshape(nskv, 128).T),
            "ones": np.ones((128, 512), bf16),
            "ident": np.eye(128, dtype=bf16),
        }
        if not zero_bias:
            m["bq"] = c(np.asarray(bq, f32)[cols].reshape(NPAIR, 128).T)
            m["bk"] = c(np.asarray(bk, f32)[cols].reshape(NPAIR, 128).T)
            m["bv"] = c(np.asarray(bv, f32)[cols].reshape(1, CPG).astype(bf16))
        in_maps.append(m)
    return in_maps, nskv, zero_bias


def gather(results, bo):
    out = np.zeros((B, S, D), np.float32)
    for core in range(NCORES):
        b = core // 2
        out[b] += np.asarray(results[core]["out"], np.float32)
    out += np.asarray(bo, np.float32)[None, None, :]
    return out


def run_on_hw(in_maps, nskv, zero_bias, trace=False, trace_cores=None):
    from concourse.bass_utils import run_bass_kernel_spmd

    nc = get_nc(nskv, zero_bias)
    return run_bass_kernel_spmd(
        nc,
        in_maps,
        list(range(NCORES)),
        trace=trace,
        trace_cores=trace_cores,
    )


def kernel(q, k, v, mask, Wq, bq, Wk, bk, Wv, bv, Wo, bo):
    in_maps, nskv, zero_bias = make_in_maps(
        q, k, v, mask, Wq, bq, Wk, bk, Wv, bv, Wo, bo
    )
    for attempt in range(2):
        res = run_on_hw(in_maps, nskv, zero_bias)
        out = gather(res.results, bo)
        if np.isfinite(out).all():
            return out
    return out



# revision 67
# speedup vs baseline: 1.0110x; 1.0110x over previous
"""Multi-head attention (B=4, S=2048, D=1024, H=16, Dk=64) on 8 trn2 NeuronCores.

Sharding: core = (batch b, head-group g), b in 0..3, g in 0..1.  Each core
computes attention for its batch and its 8 heads plus the partial out
projection for its 512 rows of Wo; host sums the two partials per batch and
adds bo.

Key optimizations over the naive version:
  - Host-side key compaction: mask keys (~50% zeros) are gathered out of k/v
    before upload, so the device only scores/exps/attends over valid keys
    (padded to a multiple of 128; pad lanes get a -1e9 exp bias -> probs 0).
  - Host-side transposes + bf16 casts: q/k/v arrive as [D, S] bf16, so phase A
    needs no PE transposes, weight loads use FWL, and DMA bytes halve.
  - Scores for a head PAIR run as two concurrent row-tiled K=64 matmuls
    (partition bases 0/64) into one PSUM tile, so a single [128,1024] ACT exp
    covers both heads (ACT is the bottleneck engine; fewer+wider ACTIVATEs).
  - The ones-column in vh yields softmax denominators for free (row 64 of the
    attn PSUM); reciprocal via the fast approx DVE op; recip broadcast across
    64 partitions with a K=1 matmul; normalize straight out of PSUM.
  - Software-pipelined emission (engine queues are FIFO): scores(sk+1) is
    emitted before attn(sk); normalization of the previous pair and the out
    projection of the previous query block are emitted as PE fillers early in
    the next pair's key loop.
"""

import sys

sys.path.insert(0, "/opt/trn_rl_repo")

import numpy as np

B, S, D, H, DK = 4, 2048, 1024, 16, 64
CPG = 512          # projection columns per core (8 heads x 64)
NPAIR = 4          # head pairs per core
NDCH = D // 128    # contraction chunks for projections
NCORES = 8

_cache = {}


def _build_nc(nskv, zero_bias):
    import contextlib

    import concourse.bass as bass
    import concourse.tile as tile
    from concourse import bacc, mybir

    f32 = mybir.dt.float32
    bf16 = mybir.dt.bfloat16
    Exp = mybir.ActivationFunctionType.Exp

    skv = nskv * 128

    nc = bacc.Bacc("TRN2", target_bir_lowering=False, debug=False)

    qt_d = nc.dram_tensor("qt", [D, S], bf16, kind="ExternalInput").ap()
    kt_d = nc.dram_tensor("kt", [D, skv], bf16, kind="ExternalInput").ap()
    vt_d = nc.dram_tensor("vt", [D, skv], bf16, kind="ExternalInput").ap()
    wq_d = nc.dram_tensor("wq", [D, CPG], bf16, kind="ExternalInput").ap()
    wk_d = nc.dram_tensor("wk", [D, CPG], bf16, kind="ExternalInput").ap()
    wv_d = nc.dram_tensor("wv", [D, CPG], bf16, kind="ExternalInput").ap()
    wo_d = nc.dram_tensor("wo", [CPG, D], bf16, kind="ExternalInput").ap()
    mb_d = nc.dram_tensor("maskbias", [128, nskv], f32, kind="ExternalInput").ap()
    ones_d = nc.dram_tensor("ones", [128, 512], bf16, kind="ExternalInput").ap()
    ident_d = nc.dram_tensor("ident", [128, 128], bf16, kind="ExternalInput").ap()
    if not zero_bias:
        bq_d = nc.dram_tensor("bq", [128, NPAIR], f32, kind="ExternalInput").ap()
        bk_d = nc.dram_tensor("bk", [128, NPAIR], f32, kind="ExternalInput").ap()
        bv_d = nc.dram_tensor("bv", [1, CPG], bf16, kind="ExternalInput").ap()
    out_d = nc.dram_tensor("out", [S, D], bf16, kind="ExternalOutput").ap()

    # skv split into 512-wide column chunks for the khT projection copies
    kq_chunks = []
    o = 0
    while o < skv:
        w = min(512, skv - o)
        kq_chunks.append((o, w))
        o += w

    with tile.TileContext(nc) as tc:
        with contextlib.ExitStack() as ctx:
            # ---------- persistent tensors + constants ----------
            persist = ctx.enter_context(tc.tile_pool(name="persist", bufs=1))
            consts = ctx.enter_context(tc.tile_pool(name="consts", bufs=1))

            qhT_sb = persist.tile([128, NPAIR, S], bf16)      # [c%128, pair, sq]
            khT_sb = persist.tile([128, NPAIR, skv], bf16)    # [c%128, pair, sk]
            vh_sb = persist.tile([128, nskv, 8, 128], bf16)  # [vdims|ones|zeros]
            concatT_sb = persist.tile([128, NPAIR, S], bf16)

            ones_sb = consts.tile([1, 512], bf16)
            nc.gpsimd.memset(ones_sb, 1.0)
            swdge_warm = consts.tile([1, 16], bf16)
            nc.gpsimd.dma_start(out=swdge_warm, in_=ones_sb[0:1, 0:16])
            # wake the other two DMA queues immediately too: first packets
            # otherwise take ~6us to land after the kernel starts
            sy_warm = consts.tile([1, 16], bf16)
            nc.sync.dma_start(out=sy_warm, in_=ones_d[0:1, 0:16])
            sc_warm = consts.tile([1, 16], bf16)
            nc.scalar.dma_start(out=sc_warm, in_=ones_d[0:1, 16:32])
            lib_warm = consts.tile([2, 16], f32)
            nc.gpsimd.memset(lib_warm[0:1, :], 1.0)
            nc.gpsimd.partition_broadcast(lib_warm, lib_warm[0:1, :])
            # prewarm the ACT exp table so the first scores exp doesn't pay
            # the table load
            act_warm = consts.tile([2, 16], bf16)
            nc.scalar.activation(out=act_warm, in_=lib_warm, func=Exp, scale=0.125)
            mb_sb = consts.tile([128, nskv], f32)
            nc.gpsimd.dma_start(out=mb_sb, in_=mb_d)
            ident_sb = consts.tile([128, 128], bf16)
            nc.gpsimd.dma_start(out=ident_sb, in_=ident_d)
            wo_sb = consts.tile([128, NPAIR, D], bf16)

            if not zero_bias:
                bq_sb = consts.tile([128, NPAIR], f32)
                nc.sync.dma_start(out=bq_sb, in_=bq_d)
                bk_sb = consts.tile([128, NPAIR], f32)
                nc.sync.dma_start(out=bk_sb, in_=bk_d)
                bv_sb = consts.tile([1, CPG], bf16)
                nc.sync.dma_start(out=bv_sb, in_=bv_d)

            # ---------- phase A: projections (no transposes needed) ----------
            xpool = ctx.enter_context(tc.tile_pool(name="xpool", bufs=1))
            wpool = ctx.enter_context(tc.tile_pool(name="wpool", bufs=1))
            with contextlib.ExitStack() as actx:
                prpool = actx.enter_context(
                    tc.tile_pool(name="prpool", bufs=8, space="PSUM")
                )

                # DMA schedule: wk first on sync (kproj needs all of it), kt
                # column-chunk-major on scalar (kproj's first output chunk
                # needs cols [0:512] of every row chunk), vt early on the
                # otherwise-idle gpsimd queue, everything else behind.
                kt_sb = xpool.tile([128, NDCH, skv], bf16, tag="kt")
                wk_sb = wpool.tile([128, NDCH, CPG], bf16, tag="wk")
                vt_sb = xpool.tile([128, NDCH, skv], bf16, tag="vt")
                wv_sb = wpool.tile([128, NDCH, CPG], bf16, tag="wv")
                qt_sb = xpool.tile([128, NDCH, S], bf16, tag="qt")
                wq_sb = wpool.tile([128, NDCH, CPG], bf16, tag="wq")
                def split_q(j):
                    return nc.scalar if j < 4 else nc.gpsimd

                for j in range(NDCH):
                    nc.sync.dma_start(out=wk_sb[:, j, :], in_=wk_d[j * 128 : j * 128 + 128, :])
                for o, w in ((0, 512), (512, skv - 512)) if skv > 512 else ((0, skv),):
                    for j in range(NDCH):
                        split_q(j).dma_start(
                            out=kt_sb[:, j, o : o + w],
                            in_=kt_d[j * 128 : j * 128 + 128, o : o + w],
                        )
                for j in range(NDCH):
                    nc.sync.dma_start(out=wq_sb[:, j, :], in_=wq_d[j * 128 : j * 128 + 128, :])
                for j in range(NDCH):
                    split_q(j).dma_start(
                        out=qt_sb[:, j, 0:512], in_=qt_d[j * 128 : j * 128 + 128, 0:512]
                    )
                for j in range(NDCH):
                    split_q(j).dma_start(out=vt_sb[:, j, :], in_=vt_d[j * 128 : j * 128 + 128, :])
                # only the ones column needs initializing: the attn matmuls
                # read vh[..., 0:DK+1], so the rest of vh can stay garbage
                nc.gpsimd.memset(vh_sb[:, :, :, DK], 1.0)
                for j in range(NDCH):
                    nc.sync.dma_start(out=wv_sb[:, j, :], in_=wv_d[j * 128 : j * 128 + 128, :])
                for j in range(NDCH):
                    nc.sync.dma_start(
                        out=qt_sb[:, j, 512:1024], in_=qt_d[j * 128 : j * 128 + 128, 512:1024]
                    )
                for j in range(NPAIR):
                    nc.sync.dma_start(
                        out=wo_sb[:, j, :], in_=wo_d[j * 128 : j * 128 + 128, :]
                    )
                for j in range(NDCH):
                    nc.sync.dma_start(
                        out=qt_sb[:, j, 1024:S], in_=qt_d[j * 128 : j * 128 + 128, 1024:S]
                    )

                # HAM warmup: the PE clock-gate needs ~3.4us of sustained
                # activity to reach full clock; burn the initial DMA wait on
                # dummy matmuls over constant data so phase A starts warm
                warm_ps = prpool.tile([128, 512], f32, tag="pr", name="warmps")
                for w in range(6):
                    nc.tensor.matmul(
                        warm_ps,
                        lhsT=ones_sb[0:1, 0:128],
                        rhs=ones_sb[0:1, :],
                        start=(w == 0),
                        stop=(w == 5),
                    )

                # k projection: khT[c, sk] = Wk^T @ kT
                for o, w in kq_chunks:
                    prs = [prpool.tile([128, 512], f32, tag="pr", name=f"pr{i}") for i in range(4)]
                    for j in range(NDCH):
                        for cch in range(4):
                            nc.tensor.matmul(
                                prs[cch][:, 0:w],
                                lhsT=wk_sb[:, j, cch * 128 : cch * 128 + 128],
                                rhs=kt_sb[:, j, o : o + w],
                                start=(j == 0),
                                stop=(j == NDCH - 1),
                            )
                    for cch in range(4):
                        if zero_bias:
                            nc.vector.tensor_copy(
                                out=khT_sb[:, cch, o : o + w], in_=prs[cch][:, 0:w]
                            )
                        else:
                            nc.vector.tensor_scalar_add(
                                khT_sb[:, cch, o : o + w],
                                prs[cch][:, 0:w],
                                bk_sb[:, cch : cch + 1],
                            )

                # (v projection runs just-in-time inside phase B's block 0)
                # q projection, block 0, pairs 0-1 only (pairs 2-3 and blocks
                # 1-3 run as phase-B fillers -- attention starts sooner)
                for sqq in range(1):
                    o = sqq * 512
                    prs = [prpool.tile([128, 512], f32, tag="pr", name=f"pr{i}") for i in range(2)]
                    for j in range(NDCH):
                        for cch in range(2):
                            nc.tensor.matmul(
                                prs[cch],
                                lhsT=wq_sb[:, j, cch * 128 : cch * 128 + 128],
                                rhs=qt_sb[:, j, o : o + 512],
                                start=(j == 0),
                                stop=(j == NDCH - 1),
                            )
                    for cch in range(2):
                        if zero_bias:
                            nc.vector.tensor_copy(
                                out=qhT_sb[:, cch, o : o + 512], in_=prs[cch]
                            )
                        else:
                            nc.vector.tensor_scalar_add(
                                qhT_sb[:, cch, o : o + 512],
                                prs[cch],
                                bq_sb[:, cch : cch + 1],
                            )

            # ---------- phase B + C: attention, fused with out projection ----
            with contextlib.ExitStack() as bctx:
                probpool = bctx.enter_context(tc.tile_pool(name="probpool", bufs=4))
                dnpool = bctx.enter_context(tc.tile_pool(name="dnpool", bufs=3))
                rc32pool = bctx.enter_context(tc.tile_pool(name="rc32pool", bufs=3))
                rc16pool = bctx.enter_context(tc.tile_pool(name="rc16pool", bufs=2))
                outpool = bctx.enter_context(tc.tile_pool(name="outpool", bufs=3))
                scpool = bctx.enter_context(
                    tc.tile_pool(name="scpool", bufs=2, space="PSUM")
                )
                atpool = bctx.enter_context(
                    tc.tile_pool(name="atpool", bufs=2, space="PSUM")
                )
                auxpool = bctx.enter_context(
                    tc.tile_pool(name="auxpool", bufs=2, space="PSUM")
                )

                def emit_norm_fast_pre(atA, atB):
                    """Tail norm, part 1: atf copies + lane-parallel recip +
                    bf16 cast + reshape back to rows.  DMAs ride the idle
                    sync queue (gpsimd still owes norm(14)'s work)."""
                    heads = ((0, atA), (1, atB))
                    atfs, rfulls = {}, {}
                    for hh, at in heads:
                        atf = rc32pool.tile([128, 512], f32, tag="atf")
                        nc.vector.tensor_copy(out=atf[0:65, :], in_=at[0:65, :])
                        atfs[hh] = atf
                    r4bs = {}
                    for hh, at in heads:
                        dn4 = dnpool.tile([4, 128], f32, tag="dn4")
                        nc.sync.dma_start(out=dn4, in_=atfs[hh][64:65, :])
                        r4 = dnpool.tile([4, 128], f32, tag="r4")
                        nc.vector.reciprocal(r4, dn4)
                        r4b = dnpool.tile([4, 128], bf16, tag="r4b")
                        nc.vector.tensor_copy(out=r4b, in_=r4)
                        r4bs[hh] = r4b
                    for hh, at in heads:
                        rfull = dnpool.tile([1, 512], bf16, tag="rfull")
                        nc.sync.dma_start(out=rfull, in_=r4bs[hh])
                        rfulls[hh] = rfull
                    return atfs, rfulls

                def emit_norm_fast_post(sqb, pair, atfs, rfulls):
                    """Tail norm, part 2: 64-partition broadcast as a K=1
                    matmul on the (tail-idle) PE, then the normalize mul."""
                    q0 = sqb * 512
                    reps = {}
                    for hh in (0, 1):
                        rep = scpool.tile([64, 512], f32, tag="sc", name="rep")
                        nc.tensor.matmul(
                            rep,
                            lhsT=ones_sb[0:1, 0:64],
                            rhs=rfulls[hh],
                            start=True,
                            stop=True,
                        )
                        reps[hh] = rep
                    for hh in (0, 1):
                        base = hh * 64
                        nc.vector.tensor_mul(
                            concatT_sb[base : base + 64, pair, q0 : q0 + 512],
                            atfs[hh][0:64, :],
                            reps[hh],
                        )

                def emit_norm(sqb, pair, atA, atB):
                    """Normalize both heads of a finished pair into concatT.

                    The PSUM->SBUF copy comes first so the attn PSUM slot is
                    released ~0.7us after the last attn matmul.  The [1,512]
                    denominator row is reshaped to [4,128] by DMA so the DVE
                    reciprocal runs lane-parallel (0.8us instead of 3.3us),
                    DMA'd back to a row, broadcast across 64 partitions and
                    multiplied on the otherwise-idle GPSIMD engine -- off
                    every critical queue."""
                    q0 = sqb * 512
                    heads = ((0, atA), (1, atB))
                    atfs, r4s, reps = {}, {}, {}
                    for hh, at in heads:
                        atf = rc32pool.tile([128, 512], f32, tag="atf")
                        nc.vector.tensor_copy(out=atf[0:65, :], in_=at[0:65, :])
                        atfs[hh] = atf
                    for hh, at in heads:
                        dn4 = dnpool.tile([4, 128], f32, tag="dn4")
                        nc.gpsimd.dma_start(out=dn4, in_=atfs[hh][64:65, :])
                        r4 = dnpool.tile([4, 128], f32, tag="r4")
                        nc.vector.reciprocal(r4, dn4)
                        r4s[hh] = r4
                    for hh, at in heads:
                        rfull = dnpool.tile([1, 512], f32, tag="rfull")
                        nc.gpsimd.dma_start(out=rfull, in_=r4s[hh])
                        rep = rc16pool.tile([64, 512], f32, tag="rep")
                        nc.gpsimd.partition_broadcast(rep, rfull)
                        reps[hh] = rep
                    for hh, at in heads:
                        base = hh * 64
                        nc.vector.tensor_mul(
                            concatT_sb[base : base + 64, pair, q0 : q0 + 512],
                            atfs[hh][0:64, :],
                            reps[hh],
                        )

                # staging for the last query block's partial out projection
                # (pairs 0-1 accumulated as their norms land; pairs 2-3 are
                # re-accumulated in PSUM at the tail via an identity matmul)
                stage_sb = persist.tile([128, 8, 512], bf16, name="stage")

                # ---- filler quanta: each closure emits EXACTLY ONE matmul
                # (~226ns of PE) so a filler never delays the next scores pair
                # past the ACT pace (~1.1us/step, ~340ns/step of PE slack).
                fillers = []
                aux_live = {}

                def q_out_mm(sqb, sqc, do, j, jmax=NPAIR - 1, dmaq=None):
                    def f():
                        q0 = sqb * 512 + sqc * 128
                        key = ("o", sqb, sqc, do)
                        if j == 0:
                            aux_live[key] = auxpool.tile(
                                [128, 512], f32, tag="aux", name="ops"
                            )
                        ops = aux_live[key]
                        nc.tensor.matmul(
                            ops,
                            lhsT=concatT_sb[:, j, q0 : q0 + 128],
                            rhs=wo_sb[:, j, do * 512 : do * 512 + 512],
                            start=(j == 0),
                            stop=(j == jmax),
                        )
                        if j == jmax:
                            del aux_live[key]
                            if jmax == NPAIR - 1:
                                osb = outpool.tile([128, 512], bf16, tag="osb")
                                nc.vector.tensor_copy(out=osb, in_=ops)
                                (dmaq or nc.sync).dma_start(
                                    out=out_d[
                                        q0 : q0 + 128, do * 512 : do * 512 + 512
                                    ],
                                    in_=osb,
                                )
                            else:
                                # partial for the last query block -> staging
                                nc.vector.tensor_copy(
                                    out=stage_sb[:, sqc * 2 + do, :], in_=ops
                                )
                    return f

                def final3_a(g, sqc, do):
                    """Tail out-projection chunk, part 1: reload the staged
                    pair-0/1 partial into PSUM via an identity matmul, then
                    accumulate the pair-2 term.  Runs under the norm chain."""
                    q0 = 3 * 512 + sqc * 128
                    pool, tag = (atpool, "at") if g % 2 else (auxpool, "aux")
                    ops = pool.tile([128, 512], f32, tag=tag, name="f3")
                    nc.tensor.matmul(
                        ops,
                        lhsT=ident_sb,
                        rhs=stage_sb[:, sqc * 2 + do, :],
                        start=True,
                        stop=False,
                    )
                    nc.tensor.matmul(
                        ops,
                        lhsT=concatT_sb[:, 2, q0 : q0 + 128],
                        rhs=wo_sb[:, 2, do * 512 : do * 512 + 512],
                        start=False,
                        stop=False,
                    )
                    return ops

                def final3_b(g, sqc, do, ops):
                    """Part 2: pair-3 term (waits the final norm), PSUM->SBUF
                    cast on the post-exp-idle scalar engine, store."""
                    q0 = 3 * 512 + sqc * 128
                    nc.tensor.matmul(
                        ops,
                        lhsT=concatT_sb[:, 3, q0 : q0 + 128],
                        rhs=wo_sb[:, 3, do * 512 : do * 512 + 512],
                        start=False,
                        stop=True,
                    )
                    # keep the gpsimd queue quiet here: its end-of-program
                    # drain takes ~8us and only overlaps the tail if gpsimd's
                    # instruction stream ends early
                    osb = outpool.tile([128, 512], bf16, tag="osb")
                    nc.scalar.copy(out=osb, in_=ops)
                    dmaq = (nc.sync, nc.scalar)[g % 2]
                    dmaq.dma_start(
                        out=out_d[q0 : q0 + 128, do * 512 : do * 512 + 512],
                        in_=osb,
                    )

                def g_kproj(o, w, cch):
                    """Group-atomic deferred k-projection chunk: one aux tile,
                    8 contraction matmuls, copy to khT.  Atomic so its PSUM
                    slot never interleaves with another in-flight group."""
                    def f():
                        pr = auxpool.tile([128, 512], f32, tag="aux", name="kpr")
                        for j in range(NDCH):
                            nc.tensor.matmul(
                                pr[:, 0:w],
                                lhsT=wk_sb[:, j, cch * 128 : cch * 128 + 128],
                                rhs=kt_sb[:, j, o : o + w],
                                start=(j == 0),
                                stop=(j == NDCH - 1),
                            )
                        if zero_bias:
                            nc.vector.tensor_copy(
                                out=khT_sb[:, cch, o : o + w], in_=pr[:, 0:w]
                            )
                        else:
                            nc.vector.tensor_scalar_add(
                                khT_sb[:, cch, o : o + w],
                                pr[:, 0:w],
                                bk_sb[:, cch : cch + 1],
                            )
                    return f

                def g_qproj(sqq, cch):
                    """Group-atomic deferred q-projection chunk."""
                    def f():
                        o = sqq * 512
                        pr = auxpool.tile([128, 512], f32, tag="aux", name="qpg")
                        for j in range(NDCH):
                            nc.tensor.matmul(
                                pr,
                                lhsT=wq_sb[:, j, cch * 128 : cch * 128 + 128],
                                rhs=qt_sb[:, j, o : o + 512],
                                start=(j == 0),
                                stop=(j == NDCH - 1),
                            )
                        if zero_bias:
                            nc.vector.tensor_copy(
                                out=qhT_sb[:, cch, o : o + 512], in_=pr
                            )
                        else:
                            nc.vector.tensor_scalar_add(
                                qhT_sb[:, cch, o : o + 512],
                                pr,
                                bq_sb[:, cch : cch + 1],
                            )
                    return f

                def q_qproj_mm(sqq, cch, j):
                    def f():
                        o = sqq * 512
                        key = ("q", sqq, cch)
                        if j == 0:
                            aux_live[key] = auxpool.tile(
                                [128, 512], f32, tag="aux", name="qpr"
                            )
                        pr = aux_live[key]
                        nc.tensor.matmul(
                            pr,
                            lhsT=wq_sb[:, j, cch * 128 : cch * 128 + 128],
                            rhs=qt_sb[:, j, o : o + 512],
                            start=(j == 0),
                            stop=(j == NDCH - 1),
                        )
                        if j == NDCH - 1:
                            del aux_live[key]
                            if zero_bias:
                                nc.vector.tensor_copy(
                                    out=qhT_sb[:, cch, o : o + 512], in_=pr
                                )
                            else:
                                nc.vector.tensor_scalar_add(
                                    qhT_sb[:, cch, o : o + 512],
                                    pr,
                                    bq_sb[:, cch : cch + 1],
                                )
                    return f

                # ---- flat (block, sk) software-pipelined stream ----
                blocks = [(sqb, pair) for sqb in range(4) for pair in range(NPAIR)]
                steps = [(bi, sk) for bi in range(len(blocks)) for sk in range(nskv)]
                at_tiles = [None] * len(blocks)
                probs_live = {}

                def emit_scores_exp(i):
                    bi, sk = steps[i]
                    sqb, pair = blocks[bi]
                    q0 = sqb * 512
                    sc = scpool.tile([128, 1024], f32, tag="sc")
                    nc.tensor.matmul(
                        sc[:, 0:512],
                        lhsT=khT_sb[0:64, pair, sk * 128 : sk * 128 + 128],
                        rhs=qhT_sb[0:64, pair, q0 : q0 + 512],
                        start=True,
                        stop=True,
                    )
                    nc.tensor.matmul(
                        sc[:, 512:1024],
                        lhsT=khT_sb[64:128, pair, sk * 128 : sk * 128 + 128],
                        rhs=qhT_sb[64:128, pair, q0 : q0 + 512],
                        start=True,
                        stop=True,
                    )
                    probs = probpool.tile([128, 1024], bf16, tag="probs")
                    probs_live[i] = probs
                    nc.scalar.activation(
                        out=probs,
                        in_=sc,
                        func=Exp,
                        bias=mb_sb[:, sk : sk + 1],
                        scale=0.125,
                    )

                def emit_vproj(skc):
                    """JIT v projection for key chunk skc (phase-B block 0):
                    vh[skc, c] = vT^T @ Wv."""
                    pr = auxpool.tile([128, 512], f32, tag="aux", name="vpr")
                    for j in range(NDCH):
                        if not zero_bias and j == 0:
                            nc.tensor.matmul(
                                pr,
                                lhsT=ones_sb[0:1, 0:128],
                                rhs=bv_sb[0:1, :],
                                start=True,
                                stop=False,
                            )
                        nc.tensor.matmul(
                            pr,
                            lhsT=vt_sb[:, j, skc * 128 : skc * 128 + 128],
                            rhs=wv_sb[:, j, :],
                            start=(zero_bias and j == 0),
                            stop=(j == NDCH - 1),
                        )
                    nc.vector.tensor_copy(
                        out=vh_sb[:, skc, :, 0:DK],
                        in_=pr.rearrange("p (h d) -> p h d", h=8),
                    )

                def emit_attn(i):
                    bi, sk = steps[i]
                    sqb, pair = blocks[bi]
                    hA, hB = 2 * pair, 2 * pair + 1
                    if sk == 0:
                        at_tiles[bi] = (
                            atpool.tile([128, 512], f32, tag="at", name="atA"),
                            atpool.tile([128, 512], f32, tag="at", name="atB"),
                        )
                    atA, atB = at_tiles[bi]
                    probs = probs_live.pop(i)
                    nc.tensor.matmul(
                        atA[0 : DK + 1, :],
                        lhsT=vh_sb[:, sk, hA, 0 : DK + 1],
                        rhs=probs[:, 0:512],
                        start=(sk == 0),
                        stop=(sk == nskv - 1),
                    )
                    nc.tensor.matmul(
                        atB[0 : DK + 1, :],
                        lhsT=vh_sb[:, sk, hB, 0 : DK + 1],
                        rhs=probs[:, 512:1024],
                        start=(sk == 0),
                        stop=(sk == nskv - 1),
                    )

                def on_attn_done(bi):
                    sqb, pair = blocks[bi]
                    if bi == len(blocks) - 1:
                        return  # tail: fast norm handled in the drain
                    emit_norm(sqb, pair, *at_tiles[bi])
                    if pair == NPAIR - 1 and sqb < 3:
                        for sqc in range(4):
                            for do in range(2):
                                for j in range(NPAIR):
                                    fillers.append(q_out_mm(sqb, sqc, do, j))
                    if bi == len(blocks) - 3:
                        # norm(13) emitted: pairs 0-1 of sqb3 are final;
                        # start their out-projection partial into staging
                        for sqc in range(4):
                            for do in range(2):
                                for j in range(2):
                                    fillers.append(
                                        q_out_mm(3, sqc, do, j, jmax=1)
                                    )
                    # (pair-2 terms of the last query block run in the drain,
                    # after norm15's vector ops are queued)

                LOOK = 2
                for i in range(len(steps)):
                    bi, sk = steps[i]
                    if i == 0:
                        for cch in (2, 3):
                            for j in range(NDCH):
                                fillers.append(q_qproj_mm(0, cch, j))
                    if sk == 0 and (
                        bi == 1 or (bi % NPAIR == 0 and blocks[bi][0] in (1, 2))
                    ):
                        sqq = 1 if bi == 1 else blocks[bi][0] + 1
                        for cch in range(4):
                            for j in range(NDCH):
                                fillers.append(q_qproj_mm(sqq, cch, j))
                    emit_scores_exp(i)
                    if bi == 0:
                        # JIT v projection paces block 0 (ACT follows behind)
                        emit_vproj(sk)
                    if i >= LOOK:
                        emit_attn(i - LOOK)
                        bj, skj = steps[i - LOOK]
                        if skj == nskv - 1:
                            on_attn_done(bj)
                    remaining = len(steps) - i
                    npop = 2 if len(fillers) > min(24, remaining) else 1
                    if bi == 0:
                        npop = 1
                    for _ in range(npop):
                        if fillers:
                            fillers.pop(0)()

                # ---- drain ----
                for i in range(max(len(steps) - LOOK, 0), len(steps)):
                    emit_attn(i)
                    bj, skj = steps[i]
                    if skj == nskv - 1:
                        on_attn_done(bj)
                # final norm's vector/gpsimd chain first; PE fillers run
                # underneath it, then the PE-side broadcast + mul, then the
                # last 8 out-projection chunks (pair-3 term + staged partial)
                atfs, rfulls = emit_norm_fast_pre(*at_tiles[len(blocks) - 1])
                while fillers:
                    fillers.pop(0)()
                chunks = [(sqc, do) for sqc in range(4) for do in range(2)]
                ops_live = {}
                # first 4 chunks' stage-reload + pair-2 terms run under the
                # norm chain (at/aux pools give 4 tiles in flight)
                for g in range(4):
                    ops_live[g] = final3_a(g, *chunks[g])
                # a few warm matmuls bridge to the reciprocal/reshape window
                # so HAM stays at full clock
                for w in range(8):
                    warm = scpool.tile([128, 1024], f32, tag="sc", name=f"wm{w}")
                    nc.tensor.matmul(
                        warm[:, 0:512],
                        lhsT=khT_sb[0:64, 0, 0:128],
                        rhs=qhT_sb[0:64, 0, 0:512],
                        start=True,
                        stop=True,
                    )
                emit_norm_fast_post(3, 3, atfs, rfulls)
                for g in range(8):
                    final3_b(g, *chunks[g], ops_live.pop(g))
                    if g + 4 < 8:
                        ops_live[g + 4] = final3_a(g + 4, *chunks[g + 4])

    nc.compile()
    return nc


def get_nc(nskv=9, zero_bias=True):
    key = (nskv, zero_bias)
    if key not in _cache:
        _cache[key] = _build_nc(nskv, zero_bias)
    return _cache[key]


def make_in_maps(q, k, v, mask, Wq, bq, Wk, bk, Wv, bv, Wo, bo):
    import ml_dtypes

    f32 = np.float32
    bf16 = ml_dtypes.bfloat16
    c = np.ascontiguousarray

    mask = np.asarray(mask)
    idxs = [np.nonzero(mask[b, 0] != 0)[0] for b in range(B)]
    kvs = [len(ix) for ix in idxs]
    nskv = max(1, (max(kvs) + 127) // 128)
    skv = nskv * 128

    zero_bias = (
        not np.any(np.asarray(bq))
        and not np.any(np.asarray(bk))
        and not np.any(np.asarray(bv))
    )

    Wq, Wk, Wv, Wo = (np.asarray(a, f32) for a in (Wq, Wk, Wv, Wo))

    in_maps = []
    for core in range(NCORES):
        b, g = core // 2, core % 2
        cols = slice(g * CPG, (g + 1) * CPG)
        ix = idxs[b]
        kv = kvs[b]

        kc = np.zeros((skv, D), f32)
        vc = np.zeros((skv, D), f32)
        kc[:kv] = np.asarray(k[b], f32)[ix]
        vc[:kv] = np.asarray(v[b], f32)[ix]

        mbflat = np.where(np.arange(skv) < kv, 0.0, -1e9).astype(f32)

        m = {
            "qt": c(np.asarray(q[b], f32).T.astype(bf16)),
            "kt": c(kc.T.astype(bf16)),
            "vt": c(vc.T.astype(bf16)),
            "wq": c(Wq[:, cols].astype(bf16)),
            "wk": c(Wk[:, cols].astype(bf16)),
            "wv": c(Wv[:, cols].astype(bf16)),
            "wo": c(Wo[cols, :].astype(bf16)),
            "maskbias": c(mbflat.reshape(nskv, 128).T),
            "ones": np.ones((128, 512), bf16),
            "ident": np.eye(128, dtype=bf16),
        }
        if not zero_bias:
            m["bq"] = c(np.asarray(bq, f32)[cols].reshape(NPAIR, 128).T)
            m["bk"] = c(np.asarray(bk, f32)[cols].reshape(NPAIR, 128).T)
            m["bv"] = c(np.asarray(bv, f32)[cols].reshape(1, CPG).astype(bf16))
        in_maps.append(m)
    return in_maps, nskv, zero_bias


def gather(results, bo):
    out = np.zeros((B, S, D), np.float32)
    for core in range(NCORES):
        b = core // 2
        out[b] += np.asarray(results[core]["out"], np.float32)
    out += np.asarray(bo, np.float32)[None, None, :]
    return out


def run_on_hw(in_maps, nskv, zero_bias, trace=False, trace_cores=None):
    from concourse.bass_utils import run_bass_kernel_spmd

    nc = get_nc(nskv, zero_bias)
    return run_bass_kernel_spmd(
        nc,
        in_maps,
        list(range(NCORES)),
        trace=trace,
        trace_cores=trace_cores,
    )


def kernel(q, k, v, mask, Wq, bq, Wk, bk, Wv, bv, Wo, bo):
    in_maps, nskv, zero_bias = make_in_maps(
        q, k, v, mask, Wq, bq, Wk, bk, Wv, bv, Wo, bo
    )
    for attempt in range(2):
        res = run_on_hw(in_maps, nskv, zero_bias)
        out = gather(res.results, bo)
        if np.isfinite(out).all():
            return out
    return out

